# revision 1
# baseline (speedup 1.0000x reference)
"""Sliding-window context attention kernel for Trainium2 (Bass/Tile).

Problem: he [B=8, T=2048, F=64]; q = he @ W + b; each step i attends to the
previous L=48 steps (step 0 attends to itself only); softmax over the window;
ctx = weighted sum of window values (values == he).

Sharding: data-parallel over batch — one batch element per NeuronCore, 8
cores, no collectives.

Per-core algorithm (chunk-major S^T formulation):
  - he loaded naturally as 16 [128, 64] tiles; heT [64, 2048] built with 16
    PE transposes.
  - qT = W.T @ heT + b  (fp32r matmuls, free dim 512).
  - For key-chunk c (keys [128c, 128c+128)): S^T = heT_chunk.T @ qT slice of
    256 queries [128c, 128c+256)  — covers every query that can see these
    keys. fp32r, free dim 256.
  - softmax without max-subtraction (max |score| ~58 for this distribution,
    exp stays finite in f32): exp on ScalarE (psum -> sbuf bf16), then one
    multiplicative band-mask (bf16) on VectorE.
  - PV: per query tile t, psum[128, 65] accumulates bf16 matmuls over the two
    contributing key chunks; V carries an extra ones-column so column 64
    accumulates the softmax denominator.
  - normalize with one strided reciprocal + one broadcast multiply per group
    of 8 tiles.
"""

import os
import numpy as np

T, F, L = 2048, 64, 48
NT = T // 128          # 16 query/key tiles
NCORES = 8

_CACHE: dict = {}


def _build_module():
    import concourse.bass as bass
    import concourse.bacc as bacc
    import concourse.tile as tile
    import concourse.mybir as mybir
    from concourse.masks import make_identity

    f32 = mybir.dt.float32
    f32r = mybir.dt.float32r
    bf16 = mybir.dt.bfloat16
    Act = mybir.ActivationFunctionType
    Alu = mybir.AluOpType

    nc = bacc.Bacc("TRN2", target_bir_lowering=False)

    hep = nc.declare_dram_parameter("he", [T, F], f32, isOutput=False)
    vbp = nc.declare_dram_parameter("vb", [T, F + 1], bf16, isOutput=False)
    Wp = nc.declare_dram_parameter("W", [F, F], f32, isOutput=False)
    bp = nc.declare_dram_parameter("bvec", [F, 1], f32, isOutput=False)
    outp = nc.declare_dram_parameter("out", [T, F], f32, isOutput=True)

    with tile.TileContext(nc) as tc:
        with (
            tc.tile_pool(name="sb", bufs=1) as sb,
            tc.tile_pool(name="ps", bufs=2, space="PSUM") as ps,
        ):
            he_nat = sb.tile([128, NT, F], f32)
            vbt = sb.tile([128, NT, F + 1], bf16)
            heT = sb.tile([64, T], f32r)
            qT = sb.tile([64, T + 128], f32r)
            eT = sb.tile([128, NT * 256], bf16)
            out_all = sb.tile([128, NT, F], f32)
            ident = sb.tile([128, 128], f32)
            maskS = sb.tile([128, 256], bf16)
            mask0 = sb.tile([128, 256], bf16)
            W_sb = sb.tile([64, F], f32r)
            W_f = sb.tile([64, F], f32)
            b_sb = sb.tile([64, 1], f32)
            rs = sb.tile([128, NT], f32)

            # ---- init: identity for PE transpose, band masks, qT padding ----
            make_identity(nc, ident)
            # valid iff 1 <= n - p <= 48  (key p, query n within the chunk)
            nc.gpsimd.memset(maskS, 1.0)
            nc.gpsimd.affine_select(
                out=maskS, in_=maskS, compare_op=Alu.is_ge, fill=0.0,
                base=-1, channel_multiplier=-1, pattern=[[1, 256]],
            )
            nc.gpsimd.affine_select(
                out=maskS, in_=maskS, compare_op=Alu.is_ge, fill=0.0,
                base=L, channel_multiplier=1, pattern=[[-1, 256]],
            )
            # chunk-0 mask additionally lets query 0 see key 0 (step 0
            # attends to itself in the reference)
            nc.gpsimd.tensor_copy(out=mask0, in_=maskS)
            nc.gpsimd.memset(mask0[0:1, 0:1], 1.0)
            nc.vector.memset(qT[:, T:T + 128].bitcast(f32), 0.0)

            # ---- input DMAs ----
            nc.sync.dma_start(out=W_f, in_=Wp[:])
            nc.vector.tensor_copy(out=W_sb, in_=W_f)
            nc.sync.dma_start(out=b_sb, in_=bp[:])
            for k in range(NT):
                nc.sync.dma_start(out=he_nat[:, k, :], in_=hep[128 * k:128 * (k + 1), :])
                nc.sync.dma_start(out=vbt[:, k, :], in_=vbp[128 * k:128 * (k + 1), :])

            # ---- heT via PE transposes (4 per psum slot, then one copy) ----
            for s in range(4):
                tr = ps.tile([64, 512], f32, tag="a", name=f"tr{s}")
                for i in range(4):
                    k = 4 * s + i
                    nc.tensor.transpose(tr[:, 128 * i:128 * (i + 1)], he_nat[:, k, :], ident)
                if s % 2 == 0:
                    nc.vector.tensor_copy(out=heT[:, 512 * s:512 * (s + 1)], in_=tr)
                else:
                    nc.scalar.copy(out=heT[:, 512 * s:512 * (s + 1)], in_=tr)

            # ---- qT = W.T @ heT + b ----
            for j in range(4):
                qp = ps.tile([64, 512], f32, tag="b", name=f"qp{j}")
                nc.tensor.matmul(
                    qp[:, :],
                    W_sb[:, :],
                    heT[:, 512 * j:512 * (j + 1)],
                )
                if j % 2 == 0:
                    nc.vector.tensor_scalar_add(
                        out=qT[:, 512 * j:512 * (j + 1)], in0=qp, scalar1=b_sb)
                else:
                    nc.scalar.activation(
                        out=qT[:, 512 * j:512 * (j + 1)], in_=qp,
                        func=Act.Identity, bias=b_sb, scale=1.0)

            def sc_group(g):
                """Scores + exp + mask for key chunks 4g..4g+3."""
                sT = ps.tile([128, 1024], f32, tag="b", name=f"sT{g}")
                for cc in range(4):
                    c = 4 * g + cc
                    nc.tensor.matmul(
                        sT[:, 256 * cc:256 * (cc + 1)],
                        heT[:, 128 * c:128 * (c + 1)],
                        qT[:, 128 * c:128 * c + 256],
                    )
                nc.scalar.activation(
                    out=eT[:, 1024 * g:1024 * (g + 1)], in_=sT[:, :], func=Act.Exp)
                # multiplicative band mask (bf16): broadcast the [128, 256]
                # mask across chunks via a 0-step middle dim
                def bmask(msk, reps):
                    return type(msk)(
                        tensor=msk.tensor, offset=msk.offset,
                        ap=[list(msk.ap[0]), [0, reps], [1, 256]])
                import concourse.bass as bass_mod
                if g == 0:
                    nc.vector.tensor_mul(out=eT[:, 0:256], in0=eT[:, 0:256], in1=mask0)
                    e13 = eT[:, 256:1024].rearrange("p (a n) -> p a n", n=256)
                    nc.vector.tensor_mul(out=e13, in0=e13, in1=bass_mod.AP(
                        tensor=maskS.tensor, offset=maskS.offset,
                        ap=[list(maskS.ap[0]), [0, 3], [1, 256]]))
                else:
                    eg = eT[:, 1024 * g:1024 * (g + 1)].rearrange("p (a n) -> p a n", n=256)
                    nc.vector.tensor_mul(out=eg, in0=eg, in1=bass_mod.AP(
                        tensor=maskS.tensor, offset=maskS.offset,
                        ap=[list(maskS.ap[0]), [0, 4], [1, 256]]))

            def pv_group(h):
                """PV matmuls + normalize for query tiles 8h..8h+7."""
                import concourse.bass as bass_mod
                ctxA = ps.tile([128, 1024], f32, tag="a", name=f"ctx{h}")
                for tt in range(8):
                    t = 8 * h + tt
                    oslice = ctxA[:, 128 * tt:128 * tt + (F + 1)]
                    if t == 0:
                        nc.tensor.matmul(
                            oslice, eT[:, 0:128], vbt[:, 0, :],
                            start=True, stop=True)
                    else:
                        nc.tensor.matmul(
                            oslice, eT[:, 256 * t:256 * t + 128], vbt[:, t, :],
                            start=True, stop=False)
                        nc.tensor.matmul(
                            oslice, eT[:, 256 * (t - 1) + 128:256 * t], vbt[:, t - 1, :],
                            start=False, stop=True)
                # reciprocal of the denominators (column 64 of each tile)
                rin = bass_mod.AP(
                    tensor=ctxA.tensor, offset=ctxA.offset + F,
                    ap=[list(ctxA.ap[0]), [128, 8]])
                nc.vector.reciprocal(out=rs[:, 8 * h:8 * h + 8], in_=rin)
                cview = ctxA.rearrange("p (k f) -> p k f", k=8)[:, :, 0:F]
                rb = bass_mod.AP(
                    tensor=rs.tensor, offset=rs.offset + 8 * h,
                    ap=[list(rs.ap[0]), [1, 8], [0, F]])
                nc.vector.tensor_mul(out=out_all[:, 8 * h:8 * h + 8, :], in0=cview, in1=rb)
                for tt in range(8):
                    t = 8 * h + tt
                    nc.sync.dma_start(
                        out=outp[128 * t:128 * (t + 1), :], in_=out_all[:, t, :])

            sc_group(0)
            sc_group(1)
            sc_group(2)
            pv_group(0)
            sc_group(3)
            pv_group(1)

    nc.compile()
    return nc


def _get_module():
    if "nc" not in _CACHE:
        _CACHE["nc"] = _build_module()
    return _CACHE["nc"]


def _run(inputs, trace=False):
    import ml_dtypes
    from concourse.bass_utils import run_bass_kernel_spmd

    he = np.ascontiguousarray(np.asarray(inputs["he"], dtype=np.float32))
    W = np.ascontiguousarray(np.asarray(inputs["W"], dtype=np.float32))
    b = np.ascontiguousarray(np.asarray(inputs["b"], dtype=np.float32))
    B = he.shape[0]
    assert he.shape == (B, T, F) and B == NCORES, he.shape

    vb = np.concatenate(
        [he, np.ones((B, T, 1), np.float32)], axis=2).astype(ml_dtypes.bfloat16)
    b2 = b.reshape(F, 1)

    nc = _get_module()
    in_maps = [
        {"he": he[c], "vb": vb[c], "W": W, "bvec": b2} for c in range(NCORES)
    ]
    try:
        res = run_bass_kernel_spmd(
            nc, in_maps, core_ids=list(range(NCORES)), trace=trace)
    except ModuleNotFoundError:
        # no NTFF profiling hook in this environment — run untraced
        res = run_bass_kernel_spmd(
            nc, in_maps, core_ids=list(range(NCORES)), trace=False)
    out = np.stack([res.results[c]["out"] for c in range(NCORES)], axis=0)
    return out.astype(np.float32), res


def kernel(**inputs) -> np.ndarray:
    out, _ = _run(inputs, trace=bool(int(os.environ.get("KERNEL_TRACE", "0"))))
    return out

